# revision 4
# baseline (speedup 1.0000x reference)
"""Trainium2 Bass kernel for BINLayer: tanh(sign(x) @ sign(W) + bias).

Full shapes: x [524288, 128] f32, W [128, 128] f32, bias [128] f32.
Data-parallel over the batch axis across 8 NeuronCores; W/bias replicated.

Per-core pipeline (65536 rows), per [128, 2048] tile (16 consecutive rows
per partition -> 8KB-contiguous DMA descriptors):
  DMA in (f32) -> PE transpose 128x128 blocks (f32, SBUF->PSUM)
  -> ACT Sign (PSUM f32 -> SBUF bf16)   [sign commutes with transpose]
  -> PE matmul: lhsT = sign(x)^T block (bf16 stationary),
                rhs  = sign(W) (bf16 moving), accumulate K=1 ones^T x bias
  -> ACT Tanh (PSUM f32 -> SBUF f32) -> DMA out.
sign values are +-1/0: exact in bf16, and the 128-term dot products are
integers, exact in f32 PSUM accumulation.
"""

import sys

if "/opt/trn_rl_repo" not in sys.path:
    sys.path.insert(0, "/opt/trn_rl_repo")

import numpy as np

B, D = 524288, 128
N_CORES = 8
B_CORE = B // N_CORES  # 65536

_CACHE = {}


def build_bass(b_core: int, rows_per_part: int = 16, reps: int = 1):
    """Build + compile the single-core Bass program for a b_core-row shard.

    reps > 1 wraps the whole computation in an on-device For_i loop that
    re-runs it reps times (same DRAM buffers) — used only for wall-clock
    HW timing, since this environment has no NTFF profiling hook.
    """
    import concourse.bass as bass  # noqa: F401
    import concourse.mybir as mybir
    from concourse import bacc
    from concourse.masks import make_identity
    from concourse.tile import TileContext

    f32 = mybir.dt.float32
    bf16 = mybir.dt.bfloat16

    tile_rows = 128 * rows_per_part
    assert b_core % tile_rows == 0
    n_tiles = b_core // tile_rows
    free_w = rows_per_part * D  # free width of one SBUF tile

    nc = bacc.Bacc("TRN2", target_bir_lowering=False, debug=False)

    x = nc.dram_tensor("x", [b_core, D], f32, kind="ExternalInput")
    w = nc.dram_tensor("w", [D, D], f32, kind="ExternalInput")
    b = nc.dram_tensor("b", [D], f32, kind="ExternalInput")
    y = nc.dram_tensor("y", [b_core, D], f32, kind="ExternalOutput")

    # row index = t*tile_rows + p*rows_per_part + r ; free index = r*D + d
    x_t = x.ap().rearrange("(t p r) d -> t p (r d)", p=128, r=rows_per_part)
    y_t = y.ap().rearrange("(t p r) d -> t p (r d)", p=128, r=rows_per_part)

    with TileContext(nc) as tc:
        with (
            tc.tile_pool(name="const", bufs=1) as cpool,
            tc.tile_pool(name="xin", bufs=3) as xpool,
            tc.tile_pool(name="xt", bufs=3) as xtpool,
            tc.tile_pool(name="out", bufs=3) as opool,
            tc.tile_pool(name="pst", bufs=3, space="PSUM") as pst_pool,
            tc.tile_pool(name="pso", bufs=3, space="PSUM") as pso_pool,
        ):
            # --- constants ---
            ident = cpool.tile([128, 128], f32)
            make_identity(nc, ident)

            w_sb = cpool.tile([128, 128], f32)
            nc.sync.dma_start(out=w_sb, in_=w.ap())
            ws_bf = cpool.tile([128, 128], bf16)
            nc.scalar.sign(out=ws_bf, in_=w_sb)

            ones_bf = cpool.tile([1, 128], bf16)
            nc.gpsimd.memset(ones_bf, 1.0)
            bias_bf = cpool.tile([1, 128], bf16)
            # SWDGE dma casts f32 -> bf16 on the fly
            nc.gpsimd.dma_start(out=bias_bf, in_=b.ap()[None, :])

            # --- main loop ---
            n_chunks = free_w // 512  # 512-col chunks (one PSUM bank each)
            from contextlib import ExitStack

            rep_ctx = ExitStack()
            if reps > 1:
                rep_ctx.enter_context(tc.For_i(0, reps, 1))
            for i in range(n_tiles):
                x_sb = xpool.tile([128, free_w], f32)
                nc.sync.dma_start(out=x_sb, in_=x_t[i])
                out_sb = opool.tile([128, free_w], f32)

                for c in range(n_chunks):
                    ps_t = pst_pool.tile([128, 512], f32)
                    for j in range(4):
                        q = c * 4 + j
                        nc.tensor.transpose(
                            ps_t[:, j * 128 : (j + 1) * 128],
                            x_sb[:, q * 128 : (q + 1) * 128],
                            ident,
                        )
                    xt_sb = xtpool.tile([128, 512], bf16)
                    nc.scalar.sign(out=xt_sb, in_=ps_t)

                    ps_o = pso_pool.tile([128, 512], f32)
                    for j in range(4):
                        blk = slice(j * 128, (j + 1) * 128)
                        nc.tensor.matmul(
                            ps_o[:, blk],
                            lhsT=xt_sb[:, blk],
                            rhs=ws_bf,
                            start=True,
                            stop=False,
                        )
                        nc.tensor.matmul(
                            ps_o[:, blk],
                            lhsT=ones_bf,
                            rhs=bias_bf,
                            start=False,
                            stop=True,
                        )
                    nc.scalar.activation(
                        out=out_sb[:, c * 512 : (c + 1) * 512],
                        in_=ps_o,
                        func=mybir.ActivationFunctionType.Tanh,
                    )

                nc.sync.dma_start(out=y_t[i], in_=out_sb)

            rep_ctx.close()

    nc.compile()
    return nc


def _get_nc(b_core: int):
    if b_core not in _CACHE:
        _CACHE[b_core] = build_bass(b_core)
    return _CACHE[b_core]


def run_spmd(nc, in_maps, **kwargs):
    from concourse.bass_utils import run_bass_kernel_spmd

    return run_bass_kernel_spmd(nc, in_maps, core_ids=list(range(len(in_maps))), **kwargs)


def kernel(inputs: np.ndarray, kernel: np.ndarray, bias: np.ndarray) -> np.ndarray:
    x = np.ascontiguousarray(np.asarray(inputs, dtype=np.float32))
    w = np.ascontiguousarray(np.asarray(kernel, dtype=np.float32))
    b = np.ascontiguousarray(np.asarray(bias, dtype=np.float32))
    assert x.shape == (B, D) and w.shape == (D, D) and b.shape == (D,)

    nc = _get_nc(B_CORE)
    in_maps = [
        {"x": x[i * B_CORE : (i + 1) * B_CORE], "w": w, "b": b}
        for i in range(N_CORES)
    ]
    res = run_spmd(nc, in_maps)
    return np.concatenate([r["y"] for r in res.results], axis=0)


# revision 6
# speedup vs baseline: 1.2220x; 1.2220x over previous
"""Trainium2 Bass kernel for BINLayer: tanh(sign(x) @ sign(W) + bias).

Full shapes: x [524288, 128] f32, W [128, 128] f32, bias [128] f32.
Data-parallel over the batch axis across 8 NeuronCores; W/bias replicated.

Per-core pipeline (65536 rows), per [128, 2048] tile (16 consecutive rows
per partition -> 8KB-contiguous DMA descriptors):
  DMA in (f32) -> PE transpose 128x128 blocks (f32, SBUF->PSUM)
  -> ACT Sign (PSUM f32 -> SBUF bf16)   [sign commutes with transpose]
  -> PE matmul: lhsT = sign(x)^T block (bf16 stationary),
                rhs  = sign(W) (bf16 moving), accumulate K=1 ones^T x bias
  -> ACT Tanh (PSUM f32 -> SBUF f32) -> DMA out.
sign values are +-1/0: exact in bf16, and the 128-term dot products are
integers, exact in f32 PSUM accumulation.
"""

import sys

if "/opt/trn_rl_repo" not in sys.path:
    sys.path.insert(0, "/opt/trn_rl_repo")

import numpy as np

B, D = 524288, 128
N_CORES = 8
B_CORE = B // N_CORES  # 65536

_CACHE = {}


def build_bass(b_core: int, rows_per_part: int = 16, reps: int = 1):
    """Build + compile the single-core Bass program for a b_core-row shard.

    reps > 1 wraps the whole computation in an on-device For_i loop that
    re-runs it reps times (same DRAM buffers) — used only for wall-clock
    HW timing, since this environment has no NTFF profiling hook.
    """
    import concourse.bass as bass  # noqa: F401
    import concourse.mybir as mybir
    from concourse import bacc
    from concourse.masks import make_identity
    from concourse.tile import TileContext

    f32 = mybir.dt.float32
    bf16 = mybir.dt.bfloat16

    tile_rows = 128 * rows_per_part
    assert b_core % tile_rows == 0
    n_tiles = b_core // tile_rows
    free_w = rows_per_part * D  # free width of one SBUF tile

    nc = bacc.Bacc("TRN2", target_bir_lowering=False, debug=False)

    x = nc.dram_tensor("x", [b_core, D], f32, kind="ExternalInput")
    w = nc.dram_tensor("w", [D, D], f32, kind="ExternalInput")
    b = nc.dram_tensor("b", [D], f32, kind="ExternalInput")
    y = nc.dram_tensor("y", [b_core, D], f32, kind="ExternalOutput")

    # row index = t*tile_rows + p*rows_per_part + r ; free index = r*D + d
    x_t = x.ap().rearrange("(t p r) d -> t p (r d)", p=128, r=rows_per_part)
    y_t = y.ap().rearrange("(t p r) d -> t p (r d)", p=128, r=rows_per_part)

    with TileContext(nc) as tc:
        with (
            tc.tile_pool(name="const", bufs=1) as cpool,
            tc.tile_pool(name="xin", bufs=3) as xpool,
            tc.tile_pool(name="xs", bufs=3) as xspool,
            tc.tile_pool(name="xt", bufs=4) as xtpool,
            tc.tile_pool(name="out", bufs=3) as opool,
            tc.tile_pool(name="pst", bufs=3, space="PSUM") as pst_pool,
            tc.tile_pool(name="pso", bufs=3, space="PSUM") as pso_pool,
        ):
            # --- constants ---
            ident_bf = cpool.tile([128, 128], bf16)
            make_identity(nc, ident_bf)

            w_sb = cpool.tile([128, 128], f32)
            nc.sync.dma_start(out=w_sb, in_=w.ap())
            ws_bf = cpool.tile([128, 128], bf16)
            nc.scalar.sign(out=ws_bf, in_=w_sb)

            ones_bf = cpool.tile([1, 128], bf16)
            nc.gpsimd.memset(ones_bf, 1.0)
            bias_bf = cpool.tile([1, 128], bf16)
            # SWDGE dma casts f32 -> bf16 on the fly
            nc.gpsimd.dma_start(out=bias_bf, in_=b.ap()[None, :])
            bias_rep = cpool.tile([1, 512], bf16)
            for r in range(4):
                nc.vector.tensor_copy(
                    out=bias_rep[:, r * 128 : (r + 1) * 128], in_=bias_bf
                )

            # --- main loop ---
            n_chunks = free_w // 512  # 512-col chunks (one PSUM bank each)
            from contextlib import ExitStack

            rep_ctx = ExitStack()
            if reps > 1:
                rep_ctx.enter_context(tc.For_i(0, reps, 1))
            for i in range(n_tiles):
                x_sb = xpool.tile([128, free_w], f32)
                nc.sync.dma_start(out=x_sb, in_=x_t[i])
                # sign for the whole tile in one ACT op, cast to bf16
                xs_bf = xspool.tile([128, free_w], bf16)
                nc.scalar.sign(out=xs_bf, in_=x_sb)
                out_sb = opool.tile([128, free_w], f32)

                for c in range(n_chunks):
                    ps_t = pst_pool.tile([128, 512], bf16)
                    for j in range(4):
                        q = c * 4 + j
                        nc.tensor.transpose(
                            ps_t[:, j * 128 : (j + 1) * 128],
                            xs_bf[:, q * 128 : (q + 1) * 128],
                            ident_bf,
                        )
                    xt_sb = xtpool.tile([128, 512], bf16)
                    nc.vector.tensor_copy(out=xt_sb, in_=ps_t)

                    ps_o = pso_pool.tile([128, 512], f32)
                    for j in range(4):
                        blk = slice(j * 128, (j + 1) * 128)
                        nc.tensor.matmul(
                            ps_o[:, blk],
                            lhsT=xt_sb[:, blk],
                            rhs=ws_bf,
                            start=(j == 0),  # one accumulation group per bank
                            stop=False,
                        )
                    # one K=1 matmul adds bias to all four blocks at once
                    nc.tensor.matmul(
                        ps_o,
                        lhsT=ones_bf,
                        rhs=bias_rep,
                        start=False,
                        stop=True,
                    )
                    nc.scalar.activation(
                        out=out_sb[:, c * 512 : (c + 1) * 512],
                        in_=ps_o,
                        func=mybir.ActivationFunctionType.Tanh,
                    )

                nc.sync.dma_start(out=y_t[i], in_=out_sb)

            rep_ctx.close()

    nc.compile()
    return nc


def _get_nc(b_core: int):
    if b_core not in _CACHE:
        _CACHE[b_core] = build_bass(b_core)
    return _CACHE[b_core]


def run_spmd(nc, in_maps, **kwargs):
    from concourse.bass_utils import run_bass_kernel_spmd

    return run_bass_kernel_spmd(nc, in_maps, core_ids=list(range(len(in_maps))), **kwargs)


def kernel(inputs: np.ndarray, kernel: np.ndarray, bias: np.ndarray) -> np.ndarray:
    x = np.ascontiguousarray(np.asarray(inputs, dtype=np.float32))
    w = np.ascontiguousarray(np.asarray(kernel, dtype=np.float32))
    b = np.ascontiguousarray(np.asarray(bias, dtype=np.float32))
    assert x.shape == (B, D) and w.shape == (D, D) and b.shape == (D,)

    nc = _get_nc(B_CORE)
    in_maps = [
        {"x": x[i * B_CORE : (i + 1) * B_CORE], "w": w, "b": b}
        for i in range(N_CORES)
    ]
    res = run_spmd(nc, in_maps)
    return np.concatenate([r["y"] for r in res.results], axis=0)
